# revision 2
# baseline (speedup 1.0000x reference)
# Multi-head attention (B=4, N=2048, D=1024, H=16, DH=64, OUT=1024) on 8 TRN2 NeuronCores.
#
# Sharding: 4 groups x 2 cores. Core c -> batch c//2, head-half c%2 (8 heads).
# Wq/Wk/Wv column-split per head group, Wo row-split; host sums the two
# partial outputs per batch (row-parallel unshard). bo folded in on even cores.
#
# Per-core kernel (all compute bf16 inputs, fp32 PSUM accumulation):
#   qT/kT projections in transposed layout [head_cols(128=2 heads), rows],
#   v projection in natural layout augmented with a ones column (M=65) so the
#   attention AV matmul emits softmax denominators for free.
#   scoresT [krow, qrow] via row-tiled K=64 matmul pairs (2 heads concurrent
#   in PE row groups 0/64). exp on ScalarE over [128,2048] PSUM tiles.
#   Normalization: reciprocal of denom row + gpsimd partition_broadcast +
#   DVE multiply. Output projection contracts ctxT over head dims in PSUM.

import contextlib

import numpy as np
import ml_dtypes

B, N, D, H = 4, 2048, 1024, 16
DH = D // H
OUT = 1024
NCORES = 8
KC = D // 128     # 8 contraction chunks for projections
RC = N // 128     # 16 row chunks
QC = N // 512     # 4 qrow chunks of 512
G = 4             # head-pair groups per core (8 heads / 2)
HPC = H // 2      # heads per core

_cache = {}


def _build_module():
    import concourse.mybir as mybir
    import concourse.tile as tile
    from concourse import bacc

    bf16 = mybir.dt.bfloat16
    f32 = mybir.dt.float32
    Exp = mybir.ActivationFunctionType.Exp
    MULT = mybir.AluOpType.mult
    ADD = mybir.AluOpType.add

    nc = bacc.Bacc(trn_type="TRN2", target_bir_lowering=False)

    xt_q = nc.declare_dram_parameter("xt_q", [KC, 128, N], bf16, isOutput=False)
    xt_k = nc.declare_dram_parameter("xt_k", [KC, 128, N], bf16, isOutput=False)
    xt_v = nc.declare_dram_parameter("xt_v", [KC, 128, N], bf16, isOutput=False)
    wq_d = nc.declare_dram_parameter("wq", [KC, 128, 512], bf16, isOutput=False)
    wk_d = nc.declare_dram_parameter("wk", [KC, 128, 512], bf16, isOutput=False)
    wv_d = nc.declare_dram_parameter("wv", [KC, 128, 512], bf16, isOutput=False)
    wo_d = nc.declare_dram_parameter("wo", [G, 128, OUT], bf16, isOutput=False)
    bq_d = nc.declare_dram_parameter("bq2", [G, 128, 1], f32, isOutput=False)
    bk_d = nc.declare_dram_parameter("bk2", [G, 128, 1], f32, isOutput=False)
    bv_d = nc.declare_dram_parameter("bv_rep", [128, 512], f32, isOutput=False)
    bo_d = nc.declare_dram_parameter("bo_rep", [128, OUT], f32, isOutput=False)
    out_d = nc.declare_dram_parameter("out", [N, OUT], f32, isOutput=True)

    with tile.TileContext(nc) as tc, contextlib.ExitStack() as ctx:
        weights = ctx.enter_context(tc.tile_pool(name="weights", bufs=1))
        qkv = ctx.enter_context(tc.tile_pool(name="qkv", bufs=1))
        xt_pool = ctx.enter_context(tc.tile_pool(name="xt", bufs=10))
        expp = ctx.enter_context(tc.tile_pool(name="expp", bufs=3))
        ctxp = ctx.enter_context(tc.tile_pool(name="ctxp", bufs=1))
        small = ctx.enter_context(tc.tile_pool(name="small", bufs=4))
        outp = ctx.enter_context(tc.tile_pool(name="outp", bufs=3))
        ps_proj = ctx.enter_context(tc.tile_pool(name="ps_proj", bufs=2, space="PSUM"))
        ps_qk = ctx.enter_context(tc.tile_pool(name="ps_qk", bufs=1, space="PSUM"))
        ps_av = ctx.enter_context(tc.tile_pool(name="ps_av", bufs=2, space="PSUM"))

        # ---- weights to SBUF
        wq_sb = weights.tile([128, KC, 512], bf16, tag="wq")
        nc.sync.dma_start(wq_sb[:], wq_d.rearrange("k p n -> p k n"))
        wk_sb = weights.tile([128, KC, 512], bf16, tag="wk")
        nc.sync.dma_start(wk_sb[:], wk_d.rearrange("k p n -> p k n"))
        wv_sb = weights.tile([128, KC, 512], bf16, tag="wv")
        nc.sync.dma_start(wv_sb[:], wv_d.rearrange("k p n -> p k n"))
        wo_sb = weights.tile([128, G, OUT], bf16, tag="wo")
        nc.sync.dma_start(wo_sb[:], wo_d.rearrange("g p n -> p g n"))
        bq_sb = weights.tile([128, G, 1], f32, tag="bq")
        nc.sync.dma_start(bq_sb[:], bq_d.rearrange("g p o -> p g o"))
        bk_sb = weights.tile([128, G, 1], f32, tag="bk")
        nc.sync.dma_start(bk_sb[:], bk_d.rearrange("g p o -> p g o"))
        bv_sb = weights.tile([128, 512], f32, tag="bv")
        nc.sync.dma_start(bv_sb[:], bv_d[:])
        bo_sb = weights.tile([128, OUT], f32, tag="bo")
        nc.sync.dma_start(bo_sb[:], bo_d[:])

        # ---- V projection: v1[rc] = [v(64 cols per head) | 1] per head, bf16
        xtv = []
        for kc in range(KC):
            t = xt_pool.tile([128, N], bf16, tag="xt")
            nc.sync.dma_start(t[:], xt_v[kc])
            xtv.append(t)
        v1 = []
        for rc in range(RC):
            ps = ps_proj.tile([128, 512], f32, tag="pp")
            for kc in range(KC):
                nc.tensor.matmul(
                    ps[:],
                    xtv[kc][:, rc * 128:(rc + 1) * 128],
                    wv_sb[:, kc, :],
                    start=(kc == 0), stop=(kc == KC - 1),
                )
            t = qkv.tile([128, HPC, DH + 1], bf16, tag=f"v1_{rc}")
            nc.vector.memset(t[:], 1.0)
            nc.vector.tensor_tensor(
                t[:, :, 0:DH],
                ps.rearrange("p (h d) -> p h d", h=HPC),
                bv_sb.rearrange("p (h d) -> p h d", h=HPC),
                ADD,
            )
            v1.append(t)

        # ---- K^T projection for all 4 pair-groups: kT[g] [128(2 heads x 64), N]
        xtk = []
        for kc in range(KC):
            t = xt_pool.tile([128, N], bf16, tag="xt")
            nc.sync.dma_start(t[:], xt_k[kc])
            xtk.append(t)
        kT = []
        for g in range(G):
            t = qkv.tile([128, N], bf16, tag=f"kT_{g}")
            for qn in range(QC):
                ps = ps_proj.tile([128, 512], f32, tag="pp")
                for kc in range(KC):
                    nc.tensor.matmul(
                        ps[:],
                        wk_sb[:, kc, g * 128:(g + 1) * 128],
                        xtk[kc][:, qn * 512:(qn + 1) * 512],
                        start=(kc == 0), stop=(kc == KC - 1),
                    )
                nc.vector.tensor_scalar_add(
                    t[:, qn * 512:(qn + 1) * 512], ps[:], bk_sb[:, g, :]
                )
            kT.append(t)

        # ---- per pair-group: Q^T projection then attention
        xtq = []
        for kc in range(KC):
            t = xt_pool.tile([128, N], bf16, tag="xt")
            nc.sync.dma_start(t[:], xt_q[kc])
            xtq.append(t)

        ctxT = [
            ctxp.tile([128, N], bf16, tag=f"ctxT_{g}", name=f"ctxT_{g}")
            for g in range(G)
        ]
        for g in range(G):
            qT = qkv.tile([128, N], bf16, tag=f"qT_{g}")
            for qn in range(QC):
                ps = ps_proj.tile([128, 512], f32, tag="pp")
                for kc in range(KC):
                    nc.tensor.matmul(
                        ps[:],
                        wq_sb[:, kc, g * 128:(g + 1) * 128],
                        xtq[kc][:, qn * 512:(qn + 1) * 512],
                        start=(kc == 0), stop=(kc == KC - 1),
                    )
                nc.vector.tensor_scalar_add(
                    qT[:, qn * 512:(qn + 1) * 512], ps[:], bq_sb[:, g, :]
                )

            for qc in range(QC):
                av_lo = ps_av.tile([DH + 1, 512], f32, tag="av")
                av_hi = ps_av.tile([DH + 1, 512], f32, tag="av")
                for kc2 in range(RC // 2):
                    pp = ps_qk.tile([128, 2048], f32, tag="qk")
                    for j in range(2):
                        kcc = 2 * kc2 + j
                        # scoresT: lhsT = kT slice (K=64), row-tiled pair (heads 2g, 2g+1)
                        nc.tensor.matmul(
                            pp[:, (2 * j) * 512:(2 * j + 1) * 512],
                            kT[g][0:64, kcc * 128:(kcc + 1) * 128],
                            qT[0:64, qc * 512:(qc + 1) * 512],
                            start=True, stop=True,
                        )
                        nc.tensor.matmul(
                            pp[:, (2 * j + 1) * 512:(2 * j + 2) * 512],
                            kT[g][64:128, kcc * 128:(kcc + 1) * 128],
                            qT[64:128, qc * 512:(qc + 1) * 512],
                            start=True, stop=True,
                        )
                    eT = expp.tile([128, 2048], bf16, tag="exp")
                    nc.scalar.activation(eT[:], pp[:], Exp)
                    for j in range(2):
                        kcc = 2 * kc2 + j
                        nc.tensor.matmul(
                            av_lo[:],
                            v1[kcc][:, 2 * g, :],
                            eT[:, (2 * j) * 512:(2 * j + 1) * 512],
                            start=(kcc == 0), stop=(kcc == RC - 1),
                        )
                        nc.tensor.matmul(
                            av_hi[:],
                            v1[kcc][:, 2 * g + 1, :],
                            eT[:, (2 * j + 1) * 512:(2 * j + 2) * 512],
                            start=(kcc == 0), stop=(kcc == RC - 1),
                        )
                # normalize head lo -> ctxT[g][0:64, qc block]
                r1 = small.tile([1, 512], f32, tag="r1")
                nc.vector.reciprocal(r1[:], av_lo[DH:DH + 1, :])
                rb = small.tile([64, 512], f32, tag="rb")
                nc.gpsimd.partition_broadcast(rb[:], r1[:])
                nc.vector.tensor_tensor(
                    ctxT[g][0:64, qc * 512:(qc + 1) * 512],
                    av_lo[0:DH, :], rb[:], MULT,
                )
                # normalize head hi -> bounce tile, DMA into partitions 64:128
                r1b = small.tile([1, 512], f32, tag="r1")
                nc.vector.reciprocal(r1b[:], av_hi[DH:DH + 1, :])
                rbb = small.tile([64, 512], f32, tag="rb")
                nc.gpsimd.partition_broadcast(rbb[:], r1b[:])
                tmp = small.tile([64, 512], bf16, tag="tmp")
                nc.vector.tensor_tensor(tmp[:], av_hi[0:DH, :], rbb[:], MULT)
                nc.sync.dma_start(ctxT[g][64:128, qc * 512:(qc + 1) * 512], tmp[:])

        # ---- output projection: out = ctx @ Wo_slice (+ bo on even cores)
        for rc in range(RC):
            for ncol in range(2):
                ps = ps_proj.tile([128, 512], f32, tag="pp")
                for g in range(G):
                    nc.tensor.matmul(
                        ps[:],
                        ctxT[g][:, rc * 128:(rc + 1) * 128],
                        wo_sb[:, g, ncol * 512:(ncol + 1) * 512],
                        start=(g == 0), stop=(g == G - 1),
                    )
                ob = outp.tile([128, 512], f32, tag="ob")
                nc.vector.tensor_tensor(
                    ob[:], ps[:], bo_sb[:, ncol * 512:(ncol + 1) * 512], ADD
                )
                nc.sync.dma_start(
                    out_d[rc * 128:(rc + 1) * 128, ncol * 512:(ncol + 1) * 512], ob[:]
                )

    nc.compile()
    return nc


def _get_module():
    if "nc" not in _cache:
        _cache["nc"] = _build_module()
    return _cache["nc"]


def _shard_inputs(key, value, query, Wk, bk, Wv, bv, Wq, bq, Wo, bo):
    bf = ml_dtypes.bfloat16
    f32 = np.float32
    scale = 1.0 / np.sqrt(np.float32(DH))

    xt = {}  # per batch transposed inputs
    for b in range(B):
        xt[b] = {
            "q": np.ascontiguousarray(query[b].T).reshape(KC, 128, N).astype(bf),
            "k": np.ascontiguousarray(key[b].T).reshape(KC, 128, N).astype(bf),
            "v": np.ascontiguousarray(value[b].T).reshape(KC, 128, N).astype(bf),
        }

    in_maps = []
    for c in range(NCORES):
        b, half = divmod(c, 2)
        cols = slice(half * 512, (half + 1) * 512)
        in_maps.append({
            "xt_q": xt[b]["q"],
            "xt_k": xt[b]["k"],
            "xt_v": xt[b]["v"],
            "wq": np.ascontiguousarray(Wq[:, cols] * scale).reshape(KC, 128, 512).astype(bf),
            "wk": np.ascontiguousarray(Wk[:, cols]).reshape(KC, 128, 512).astype(bf),
            "wv": np.ascontiguousarray(Wv[:, cols]).reshape(KC, 128, 512).astype(bf),
            "wo": np.ascontiguousarray(Wo[cols, :]).reshape(G, 128, OUT).astype(bf),
            "bq2": (bq[cols] * scale).reshape(G, 128, 1).astype(f32),
            "bk2": bk[cols].reshape(G, 128, 1).astype(f32),
            "bv_rep": np.broadcast_to(bv[cols], (128, 512)).astype(f32),
            "bo_rep": (np.broadcast_to(bo, (128, OUT)).astype(f32)
                       if half == 0 else np.zeros((128, OUT), f32)),
        })
    return in_maps


def kernel(key, value, query, Wk, bk, Wv, bv, Wq, bq, Wo, bo):
    from concourse.bass_utils import run_bass_kernel_spmd

    key, value, query = np.asarray(key), np.asarray(value), np.asarray(query)
    Wk, bk, Wv, bv = np.asarray(Wk), np.asarray(bk), np.asarray(Wv), np.asarray(bv)
    Wq, bq, Wo, bo = np.asarray(Wq), np.asarray(bq), np.asarray(Wo), np.asarray(bo)

    nc = _get_module()
    in_maps = _shard_inputs(key, value, query, Wk, bk, Wv, bv, Wq, bq, Wo, bo)
    res = run_bass_kernel_spmd(nc, in_maps, core_ids=list(range(NCORES)))
    parts = [res.results[c]["out"] for c in range(NCORES)]
    out = np.empty((B, N, OUT), np.float32)
    for b in range(B):
        np.add(parts[2 * b], parts[2 * b + 1], out=out[b])
    return out


# revision 4
# speedup vs baseline: 1.0691x; 1.0691x over previous
# Multi-head attention (B=4, N=2048, D=1024, H=16, DH=64, OUT=1024) on 8 TRN2 NeuronCores.
#
# Sharding: 4 groups x 2 cores. Core c -> batch c//2, head-half c%2 (8 heads).
# Wq/Wk/Wv column-split per head group, Wo row-split; host sums the two
# partial outputs per batch (row-parallel unshard). bo folded in on even cores.
#
# Per-core kernel (all compute bf16 inputs, fp32 PSUM accumulation):
#   qT/kT projections in transposed layout [head_cols(128=2 heads), rows],
#   v projection in natural layout augmented with a ones column (M=65) so the
#   attention AV matmul emits softmax denominators for free.
#   scoresT [krow, qrow] via row-tiled K=64 matmul pairs (2 heads concurrent
#   in PE row groups 0/64). exp on ScalarE over [128,2048] PSUM tiles.
#   Normalization: reciprocal of denom row + gpsimd partition_broadcast +
#   DVE multiply. Output projection contracts ctxT over head dims in PSUM.

import contextlib

import numpy as np
import ml_dtypes

B, N, D, H = 4, 2048, 1024, 16
DH = D // H
OUT = 1024
NCORES = 8
KC = D // 128     # 8 contraction chunks for projections
RC = N // 128     # 16 row chunks
QC = N // 512     # 4 qrow chunks of 512
G = 4             # head-pair groups per core (8 heads / 2)
HPC = H // 2      # heads per core

_cache = {}


def _build_module():
    import concourse.mybir as mybir
    import concourse.tile as tile
    from concourse import bacc

    bf16 = mybir.dt.bfloat16
    f32 = mybir.dt.float32
    Exp = mybir.ActivationFunctionType.Exp
    MULT = mybir.AluOpType.mult
    ADD = mybir.AluOpType.add

    nc = bacc.Bacc(trn_type="TRN2", target_bir_lowering=False)

    xt_q = nc.declare_dram_parameter("xt_q", [KC, 128, N], bf16, isOutput=False)
    xt_k = nc.declare_dram_parameter("xt_k", [KC, 128, N], bf16, isOutput=False)
    xt_v = nc.declare_dram_parameter("xt_v", [KC, 128, N], bf16, isOutput=False)
    wq_d = nc.declare_dram_parameter("wq", [KC, 128, 512], bf16, isOutput=False)
    wk_d = nc.declare_dram_parameter("wk", [KC, 128, 512], bf16, isOutput=False)
    wv_d = nc.declare_dram_parameter("wv", [KC, 128, 512], bf16, isOutput=False)
    wo_d = nc.declare_dram_parameter("wo", [G, 128, OUT], bf16, isOutput=False)
    bq_d = nc.declare_dram_parameter("bq2", [G, 128, 1], f32, isOutput=False)
    bk_d = nc.declare_dram_parameter("bk2", [G, 128, 1], f32, isOutput=False)
    bv_d = nc.declare_dram_parameter("bv_rep", [128, 512], f32, isOutput=False)
    bo_d = nc.declare_dram_parameter("bo_rep", [128, OUT], f32, isOutput=False)
    out_d = nc.declare_dram_parameter("out", [N, OUT], f32, isOutput=True)

    with tile.TileContext(nc) as tc, contextlib.ExitStack() as ctx:
        weights = ctx.enter_context(tc.tile_pool(name="weights", bufs=1))
        qkv = ctx.enter_context(tc.tile_pool(name="qkv", bufs=1))
        xt_pool = ctx.enter_context(tc.tile_pool(name="xt", bufs=10))
        expp = ctx.enter_context(tc.tile_pool(name="expp", bufs=3))
        ctxp = ctx.enter_context(tc.tile_pool(name="ctxp", bufs=1))
        small = ctx.enter_context(tc.tile_pool(name="small", bufs=4))
        outp = ctx.enter_context(tc.tile_pool(name="outp", bufs=3))
        ps_proj = ctx.enter_context(tc.tile_pool(name="ps_proj", bufs=2, space="PSUM"))
        ps_qk = ctx.enter_context(tc.tile_pool(name="ps_qk", bufs=1, space="PSUM"))
        ps_av = ctx.enter_context(tc.tile_pool(name="ps_av", bufs=2, space="PSUM"))

        # ---- weights to SBUF
        wq_sb = weights.tile([128, KC, 512], bf16, tag="wq")
        nc.sync.dma_start(wq_sb[:], wq_d.rearrange("k p n -> p k n"))
        wk_sb = weights.tile([128, KC, 512], bf16, tag="wk")
        nc.sync.dma_start(wk_sb[:], wk_d.rearrange("k p n -> p k n"))
        wv_sb = weights.tile([128, KC, 512], bf16, tag="wv")
        nc.sync.dma_start(wv_sb[:], wv_d.rearrange("k p n -> p k n"))
        wo_sb = weights.tile([128, G, OUT], bf16, tag="wo")
        nc.sync.dma_start(wo_sb[:], wo_d.rearrange("g p n -> p g n"))
        bq_sb = weights.tile([128, G, 1], f32, tag="bq")
        nc.sync.dma_start(bq_sb[:], bq_d.rearrange("g p o -> p g o"))
        bk_sb = weights.tile([128, G, 1], f32, tag="bk")
        nc.sync.dma_start(bk_sb[:], bk_d.rearrange("g p o -> p g o"))
        bv_sb = weights.tile([128, 512], f32, tag="bv")
        nc.sync.dma_start(bv_sb[:], bv_d[:])
        bo_sb = weights.tile([128, OUT], f32, tag="bo")
        nc.sync.dma_start(bo_sb[:], bo_d[:])

        # ---- V projection: v1[rc] = [v(64 cols per head) | 1] per head, bf16
        xtv = []
        for kc in range(KC):
            t = xt_pool.tile([128, N], bf16, tag="xt")
            nc.sync.dma_start(t[:], xt_v[kc])
            xtv.append(t)
        v1 = []
        for rc in range(RC):
            ps = ps_proj.tile([128, 512], f32, tag="pp")
            for kc in range(KC):
                nc.tensor.matmul(
                    ps[:],
                    xtv[kc][:, rc * 128:(rc + 1) * 128],
                    wv_sb[:, kc, :],
                    start=(kc == 0), stop=(kc == KC - 1),
                )
            t = qkv.tile([128, HPC, DH + 1], bf16, tag=f"v1_{rc}")
            nc.vector.memset(t[:], 1.0)
            nc.vector.tensor_tensor(
                t[:, :, 0:DH],
                ps.rearrange("p (h d) -> p h d", h=HPC),
                bv_sb.rearrange("p (h d) -> p h d", h=HPC),
                ADD,
            )
            v1.append(t)

        # ---- K^T projection for all 4 pair-groups: kT[g] [128(2 heads x 64), N]
        xtk = []
        for kc in range(KC):
            t = xt_pool.tile([128, N], bf16, tag="xt")
            nc.sync.dma_start(t[:], xt_k[kc])
            xtk.append(t)
        kT = []
        for g in range(G):
            t = qkv.tile([128, N], bf16, tag=f"kT_{g}")
            for qn in range(QC):
                ps = ps_proj.tile([128, 512], f32, tag="pp")
                for kc in range(KC):
                    nc.tensor.matmul(
                        ps[:],
                        wk_sb[:, kc, g * 128:(g + 1) * 128],
                        xtk[kc][:, qn * 512:(qn + 1) * 512],
                        start=(kc == 0), stop=(kc == KC - 1),
                    )
                nc.vector.tensor_scalar_add(
                    t[:, qn * 512:(qn + 1) * 512], ps[:], bk_sb[:, g, :]
                )
            kT.append(t)

        # ---- per pair-group: Q^T projection then attention
        xtq = []
        for kc in range(KC):
            t = xt_pool.tile([128, N], bf16, tag="xt")
            nc.sync.dma_start(t[:], xt_q[kc])
            xtq.append(t)

        ctxT = [
            ctxp.tile([128, N], bf16, tag=f"ctxT_{g}", name=f"ctxT_{g}")
            for g in range(G)
        ]
        for g in range(G):
            qT = qkv.tile([128, N], bf16, tag=f"qT_{g}")
            for qn in range(QC):
                ps = ps_proj.tile([128, 512], f32, tag="pp")
                for kc in range(KC):
                    nc.tensor.matmul(
                        ps[:],
                        wq_sb[:, kc, g * 128:(g + 1) * 128],
                        xtq[kc][:, qn * 512:(qn + 1) * 512],
                        start=(kc == 0), stop=(kc == KC - 1),
                    )
                nc.vector.tensor_scalar_add(
                    qT[:, qn * 512:(qn + 1) * 512], ps[:], bq_sb[:, g, :]
                )

            for qc in range(QC):
                av_lo = ps_av.tile([DH + 1, 512], f32, tag="av")
                av_hi = ps_av.tile([DH + 1, 512], f32, tag="av")
                for kc2 in range(RC // 2):
                    pp = ps_qk.tile([128, 2048], f32, tag="qk")
                    for j in range(2):
                        kcc = 2 * kc2 + j
                        # scoresT: lhsT = kT slice (K=64), row-tiled pair (heads 2g, 2g+1)
                        nc.tensor.matmul(
                            pp[:, (2 * j) * 512:(2 * j + 1) * 512],
                            kT[g][0:64, kcc * 128:(kcc + 1) * 128],
                            qT[0:64, qc * 512:(qc + 1) * 512],
                            start=True, stop=True,
                        )
                        nc.tensor.matmul(
                            pp[:, (2 * j + 1) * 512:(2 * j + 2) * 512],
                            kT[g][64:128, kcc * 128:(kcc + 1) * 128],
                            qT[64:128, qc * 512:(qc + 1) * 512],
                            start=True, stop=True,
                        )
                    eT = expp.tile([128, 2048], bf16, tag="exp")
                    nc.scalar.activation(eT[:], pp[:], Exp)
                    for j in range(2):
                        kcc = 2 * kc2 + j
                        nc.tensor.matmul(
                            av_lo[:],
                            v1[kcc][:, 2 * g, :],
                            eT[:, (2 * j) * 512:(2 * j + 1) * 512],
                            start=(kcc == 0), stop=(kcc == RC - 1),
                        )
                        nc.tensor.matmul(
                            av_hi[:],
                            v1[kcc][:, 2 * g + 1, :],
                            eT[:, (2 * j + 1) * 512:(2 * j + 2) * 512],
                            start=(kcc == 0), stop=(kcc == RC - 1),
                        )
                # normalize head lo -> ctxT[g][0:64, qc block]
                r1 = small.tile([1, 512], f32, tag="r1")
                nc.vector.reciprocal(r1[:], av_lo[DH:DH + 1, :])
                rb = small.tile([64, 512], f32, tag="rb")
                nc.gpsimd.partition_broadcast(rb[:], r1[:])
                nc.vector.tensor_tensor(
                    ctxT[g][0:64, qc * 512:(qc + 1) * 512],
                    av_lo[0:DH, :], rb[:], MULT,
                )
                # normalize head hi -> bounce tile, DMA into partitions 64:128
                r1b = small.tile([1, 512], f32, tag="r1")
                nc.vector.reciprocal(r1b[:], av_hi[DH:DH + 1, :])
                rbb = small.tile([64, 512], f32, tag="rb")
                nc.gpsimd.partition_broadcast(rbb[:], r1b[:])
                tmp = small.tile([64, 512], bf16, tag="tmp")
                nc.vector.tensor_tensor(tmp[:], av_hi[0:DH, :], rbb[:], MULT)
                nc.sync.dma_start(ctxT[g][64:128, qc * 512:(qc + 1) * 512], tmp[:])

        # ---- output projection: out = ctx @ Wo_slice (+ bo on even cores)
        for rc in range(RC):
            for ncol in range(2):
                ps = ps_proj.tile([128, 512], f32, tag="pp")
                for g in range(G):
                    nc.tensor.matmul(
                        ps[:],
                        ctxT[g][:, rc * 128:(rc + 1) * 128],
                        wo_sb[:, g, ncol * 512:(ncol + 1) * 512],
                        start=(g == 0), stop=(g == G - 1),
                    )
                ob = outp.tile([128, 512], f32, tag="ob")
                nc.vector.tensor_tensor(
                    ob[:], ps[:], bo_sb[:, ncol * 512:(ncol + 1) * 512], ADD
                )
                nc.sync.dma_start(
                    out_d[rc * 128:(rc + 1) * 128, ncol * 512:(ncol + 1) * 512], ob[:]
                )

    nc.compile()
    return nc


def _get_module():
    if "nc" not in _cache:
        _cache["nc"] = _build_module()
    return _cache["nc"]


def _get_runner():
    """Build the PJRT executable once (mirrors bass2jax.run_bass_via_pjrt) and
    return a callable in_maps -> list of per-core output dicts."""
    if "runner" in _cache:
        return _cache["runner"]

    import jax
    import numpy as np
    import concourse.mybir as mybir
    from concourse import bass2jax
    from jax.sharding import Mesh, PartitionSpec
    from jax.experimental.shard_map import shard_map

    nc = _get_module()
    bass2jax.install_neuronx_cc_hook()

    partition_name = nc.partition_id_tensor.name if nc.partition_id_tensor else None
    in_names, out_names, out_avals, zero_outs = [], [], [], []
    for alloc in nc.m.functions[0].allocations:
        if not isinstance(alloc, mybir.MemoryLocationSet):
            continue
        name = alloc.memorylocations[0].name
        if alloc.kind == "ExternalInput":
            if name != partition_name:
                in_names.append(name)
        elif alloc.kind == "ExternalOutput":
            shape = tuple(alloc.tensor_shape)
            dtype = mybir.dt.np(alloc.dtype)
            out_names.append(name)
            out_avals.append(jax.core.ShapedArray(shape, dtype))
            zero_outs.append(np.zeros(shape, dtype))
    n_params = len(in_names)
    n_outs = len(out_avals)
    all_in_names = list(in_names) + list(out_names)
    if partition_name is not None:
        all_in_names.append(partition_name)
    donate = tuple(range(n_params, n_params + n_outs))

    def _body(*args):
        operands = list(args)
        if partition_name is not None:
            operands.append(bass2jax.partition_id_tensor())
        outs = bass2jax._bass_exec_p.bind(
            *operands,
            out_avals=tuple(out_avals),
            in_names=tuple(all_in_names),
            out_names=tuple(out_names),
            lowering_input_output_aliases=(),
            sim_require_finite=True,
            sim_require_nnan=True,
            nc=nc,
        )
        return tuple(outs)

    devices = jax.devices()[:NCORES]
    mesh = Mesh(np.asarray(devices), ("core",))
    in_specs = (PartitionSpec("core"),) * (n_params + n_outs)
    out_specs = (PartitionSpec("core"),) * n_outs
    sharded = jax.jit(
        shard_map(_body, mesh=mesh, in_specs=in_specs, out_specs=out_specs,
                  check_rep=False),
        donate_argnums=donate, keep_unused=True,
    )

    def run(in_maps):
        concat_in = [
            np.concatenate([np.asarray(in_maps[c][name]) for c in range(NCORES)], axis=0)
            for name in in_names
        ]
        concat_zeros = [
            np.zeros((NCORES * z.shape[0], *z.shape[1:]), z.dtype) for z in zero_outs
        ]
        out_arrs = sharded(*concat_in, *concat_zeros)
        return [
            {
                name: np.asarray(out_arrs[i]).reshape(NCORES, *out_avals[i].shape)[c]
                for i, name in enumerate(out_names)
            }
            for c in range(NCORES)
        ]

    _cache["runner"] = run
    return run


def _shard_inputs(key, value, query, Wk, bk, Wv, bv, Wq, bq, Wo, bo):
    bf = ml_dtypes.bfloat16
    f32 = np.float32
    scale = 1.0 / np.sqrt(np.float32(DH))

    xt = {}  # per batch transposed inputs
    for b in range(B):
        xt[b] = {
            "q": np.ascontiguousarray(query[b].T).reshape(KC, 128, N).astype(bf),
            "k": np.ascontiguousarray(key[b].T).reshape(KC, 128, N).astype(bf),
            "v": np.ascontiguousarray(value[b].T).reshape(KC, 128, N).astype(bf),
        }

    in_maps = []
    for c in range(NCORES):
        b, half = divmod(c, 2)
        cols = slice(half * 512, (half + 1) * 512)
        in_maps.append({
            "xt_q": xt[b]["q"],
            "xt_k": xt[b]["k"],
            "xt_v": xt[b]["v"],
            "wq": np.ascontiguousarray(Wq[:, cols] * scale).reshape(KC, 128, 512).astype(bf),
            "wk": np.ascontiguousarray(Wk[:, cols]).reshape(KC, 128, 512).astype(bf),
            "wv": np.ascontiguousarray(Wv[:, cols]).reshape(KC, 128, 512).astype(bf),
            "wo": np.ascontiguousarray(Wo[cols, :]).reshape(G, 128, OUT).astype(bf),
            "bq2": (bq[cols] * scale).reshape(G, 128, 1).astype(f32),
            "bk2": bk[cols].reshape(G, 128, 1).astype(f32),
            "bv_rep": np.broadcast_to(bv[cols], (128, 512)).astype(f32),
            "bo_rep": (np.broadcast_to(bo, (128, OUT)).astype(f32)
                       if half == 0 else np.zeros((128, OUT), f32)),
        })
    return in_maps


def kernel(key, value, query, Wk, bk, Wv, bv, Wq, bq, Wo, bo):
    key, value, query = np.asarray(key), np.asarray(value), np.asarray(query)
    Wk, bk, Wv, bv = np.asarray(Wk), np.asarray(bk), np.asarray(Wv), np.asarray(bv)
    Wq, bq, Wo, bo = np.asarray(Wq), np.asarray(bq), np.asarray(Wo), np.asarray(bo)

    run = _get_runner()
    in_maps = _shard_inputs(key, value, query, Wk, bk, Wv, bv, Wq, bq, Wo, bo)
    results = run(in_maps)
    parts = [results[c]["out"] for c in range(NCORES)]
    out = np.empty((B, N, OUT), np.float32)
    for b in range(B):
        np.add(parts[2 * b], parts[2 * b + 1], out=out[b])
    return out
